# revision 25
# baseline (speedup 1.0000x reference)
"""Trainium2 Bass kernel for nn_DeepTimeGraphNet (per-row conv/pool pyramid + classifier).

Contract: kernel(**inputs) takes the FULL unsharded inputs (keys as in
setup_inputs()) and returns the FULL (64, 3) softmax output.

Sharding: pure data parallel over batch. Core i handles batch rows
[8i, 8i+8) = 8192 (batch, node) rows of length 1200. Inside each core the
rows are processed as 8 "supertiles" of 1024 rows = 128 SBUF partitions x 8
column groups, so every engine instruction covers 1024 rows at once.

Per-row math (weights baked in as immediates at trace time):
conv(k2,s2) -> maxpool3+relu -> conv(k4,s2,p1) -> maxpool2+relu ->
conv(k4,s2,p1) -> maxpool2+relu -> conv(k4,s2,p1) -> maxpool2+relu ->
conv(k3,s1) -> 1 scalar per row. Then an on-device classifier
(feat @ cls_w.T + cls_b) via accumulating PE matmuls and an exact softmax.

Engine split: ScalarE takes the first (affine) tap of each conv; VectorE the
fused scalar_tensor_tensor accumulate taps and fused max-max-relu pools. The
big stages S1-S4 run per supertile; the small stages S5-S9 are batched over
supertile halves so their per-op overhead is paid once per 4096 rows. The
kernel is HBM-bound (~39.3 MB/core @ ~358 GB/s).
"""
import os
import sys

for _p in ("/root/.axon_site/_ro/trn_rl_repo", "/opt/trn_rl_repo"):
    if os.path.isdir(_p) and _p not in sys.path:
        sys.path.insert(0, _p)

import numpy as np  # noqa: E402

import concourse.bacc as bacc  # noqa: E402
import concourse.tile as tile  # noqa: E402
from concourse import mybir  # noqa: E402
from concourse.bass_utils import run_bass_kernel_spmd  # noqa: E402

F32 = mybir.dt.float32
Alu = mybir.AluOpType
Act = mybir.ActivationFunctionType

BS, NN, T = 64, 1024, 1200
N_CORES = 8
S_PER_CORE = 8          # supertiles per core; each = 1024 rows (one batch row)
C = 8                   # column groups per supertile (128 rows each)

_CACHE = {}


def _build(w):
    """Build + compile the per-core SPMD program with weights baked in."""
    nc = bacc.Bacc("TRN2", target_bir_lowering=False, debug=False)
    x = nc.dram_tensor("x", [S_PER_CORE * C * 128, T], F32, kind="ExternalInput")
    clswt = nc.dram_tensor("clswt", [128, 24], F32, kind="ExternalInput")
    out = nc.dram_tensor("out", [8, 3], F32, kind="ExternalOutput")

    w2, w4, w6, w8 = w["w2"], w["w4"], w["w6"], w["w8"]
    stt = nc.vector.scalar_tensor_tensor

    with tile.TileContext(nc) as tc:
        with (
            tc.tile_pool(name="xpool", bufs=4) as xpool,
            tc.tile_pool(name="wk", bufs=2) as wk,
            tc.tile_pool(name="const", bufs=1) as const,
            tc.tile_pool(name="psum", bufs=1, space="PSUM") as psum,
        ):
            clsw = const.tile([128, 24], F32)
            nc.sync.dma_start(clsw[:], clswt[:])
            featmat = const.tile([128, 64], F32)

            # persistent staging for the batched small stages
            r3all = const.tile([128, S_PER_CORE * C * 50], F32)
            y4all = const.tile([128, S_PER_CORE * C * 25], F32)
            r5all = const.tile([128, S_PER_CORE * C * 12], F32)
            y6all = const.tile([128, S_PER_CORE * C * 6], F32)
            r7all = const.tile([128, S_PER_CORE * C * 3], F32)
            fball = const.tile([128, S_PER_CORE * C], F32)
            r3v = r3all[:].rearrange("p (s c t) -> p s c t", s=S_PER_CORE, c=C)
            y4v = y4all[:].rearrange("p (s c t) -> p s c t", s=S_PER_CORE, c=C)
            r5v = r5all[:].rearrange("p (s c t) -> p s c t", s=S_PER_CORE, c=C)
            y6v = y6all[:].rearrange("p (s c t) -> p s c t", s=S_PER_CORE, c=C)
            r7v = r7all[:].rearrange("p (s c t) -> p s c t", s=S_PER_CORE, c=C)
            fbv = fball[:].rearrange("p (s c) -> p s c", s=S_PER_CORE)
            fmv = featmat[:].rearrange("p (s c) -> p s c", s=S_PER_CORE)

            x4 = x[:].rearrange("(s c p) t -> s p c t", s=S_PER_CORE, c=C, p=128)

            def tail_batch(lo, hi):
                """S5..S9 batched over supertiles [lo, hi)."""
                sl = slice(lo, hi)
                R3 = r3v[:, sl]
                Y4 = y4v[:, sl]
                nc.scalar.activation(Y4, R3[:, :, :, 0:50:2], Act.Copy,
                                     bias=w["b4"], scale=w4[1])
                stt(Y4, R3[:, :, :, 1:50:2], w4[2], Y4, Alu.mult, Alu.add)
                stt(Y4[:, :, :, 1:25], R3[:, :, :, 1:48:2], w4[0],
                    Y4[:, :, :, 1:25], Alu.mult, Alu.add)
                stt(Y4[:, :, :, 0:24], R3[:, :, :, 2:49:2], w4[3],
                    Y4[:, :, :, 0:24], Alu.mult, Alu.add)
                R5 = r5v[:, sl]
                stt(R5, Y4[:, :, :, 0:24:2], 0.0, Y4[:, :, :, 1:25:2],
                    Alu.max, Alu.max)
                Y6 = y6v[:, sl]
                nc.scalar.activation(Y6, R5[:, :, :, 0:12:2], Act.Copy,
                                     bias=w["b6"], scale=w6[1])
                stt(Y6, R5[:, :, :, 1:12:2], w6[2], Y6, Alu.mult, Alu.add)
                stt(Y6[:, :, :, 1:6], R5[:, :, :, 1:10:2], w6[0],
                    Y6[:, :, :, 1:6], Alu.mult, Alu.add)
                stt(Y6[:, :, :, 0:5], R5[:, :, :, 2:11:2], w6[3],
                    Y6[:, :, :, 0:5], Alu.mult, Alu.add)
                R7 = r7v[:, sl]
                stt(R7, Y6[:, :, :, 0:6:2], 0.0, Y6[:, :, :, 1:6:2],
                    Alu.max, Alu.max)
                FB = fbv[:, sl]
                nc.scalar.activation(FB, R7[:, :, :, 0], Act.Copy,
                                     bias=w["b8"], scale=w8[0])
                stt(FB, R7[:, :, :, 1], w8[1], FB, Alu.mult, Alu.add)
                stt(fmv[:, sl], R7[:, :, :, 2], w8[2], FB, Alu.mult, Alu.add)

            H = C // 2
            for s in range(S_PER_CORE):
                # S1: conv0 k2 s2 -> y0 (600). x arrives in half-supertile
                # tiles (finer slot recycling keeps the HBM stream moving);
                # s=0 additionally ramps at quarter granularity.
                y0 = wk.tile([128, C * 600], F32)
                y3 = y0[:].rearrange("p (c t) -> p c t", c=C)
                for h in range(2):
                    xh = xpool.tile([128, H * T], F32)
                    xh3 = xh[:].rearrange("p (c t) -> p c t", c=H)
                    cofs = h * H
                    dstep = 1 if (s == 0 and h == 0) else (H // 2 if s == 0 else H)
                    for c0 in range(0, H, dstep):
                        nc.sync.dma_start(xh3[:, c0:c0 + dstep],
                                          x4[s][:, cofs + c0:cofs + c0 + dstep])
                    for c0 in range(0, H, dstep):
                        yv = y3[:, cofs + c0:cofs + c0 + dstep]
                        nc.scalar.activation(yv, xh3[:, c0:c0 + dstep, 0:T:2],
                                             Act.Copy, bias=w["b0"], scale=w["w00"])
                        stt(yv, xh3[:, c0:c0 + dstep, 1:T:2], w["w01"],
                            yv, Alu.mult, Alu.add)

                # S2: maxpool3 + relu -> r1 (200)
                mx = wk.tile([128, C * 200], F32)
                mx3 = mx[:].rearrange("p (c t) -> p c t", c=C)
                nc.vector.tensor_tensor(mx3, y3[:, :, 0:600:3], y3[:, :, 1:600:3],
                                        Alu.max)
                r1 = wk.tile([128, C * 200], F32)
                r1_3 = r1[:].rearrange("p (c t) -> p c t", c=C)
                stt(r1_3, mx3, 0.0, y3[:, :, 2:600:3], Alu.max, Alu.max)

                # S3: conv2 k4 s2 p1 -> y2 (100)
                y2 = wk.tile([128, C * 100], F32)
                y2_3 = y2[:].rearrange("p (c t) -> p c t", c=C)
                nc.scalar.activation(y2_3, r1_3[:, :, 0:200:2], Act.Copy,
                                     bias=w["b2"], scale=w2[1])
                stt(y2_3, r1_3[:, :, 1:200:2], w2[2], y2_3, Alu.mult, Alu.add)
                stt(y2_3[:, :, 1:100], r1_3[:, :, 1:198:2], w2[0],
                    y2_3[:, :, 1:100], Alu.mult, Alu.add)
                stt(y2_3[:, :, 0:99], r1_3[:, :, 2:199:2], w2[3],
                    y2_3[:, :, 0:99], Alu.mult, Alu.add)

                # S4: maxpool2 + relu -> r3 (50), into the staging buffer
                stt(r3v[:, s], y2_3[:, :, 0:100:2], 0.0, y2_3[:, :, 1:100:2],
                    Alu.max, Alu.max)

                if s == 3:
                    tail_batch(0, 4)
                elif s == 6:
                    tail_batch(4, 7)
            tail_batch(7, 8)

            # classifier: logits[s, cls] = sum_c featmat[:, c::8].T @ clsw
            lg = psum.tile([8, 3], F32)
            for c in range(C):
                nc.tensor.matmul(lg[:], featmat[:, c::8], clsw[:, c * 3:(c + 1) * 3],
                                 start=(c == 0), stop=(c == C - 1))
            lgs = const.tile([8, 3], F32)
            nc.vector.tensor_copy(lgs[:], lg[:])
            for cls in range(3):
                if w["cls_b"][cls] != 0.0:
                    nc.vector.tensor_scalar_add(lgs[:, cls:cls + 1],
                                                lgs[:, cls:cls + 1], w["cls_b"][cls])
            # softmax (max-subtracted, like jax.nn.softmax)
            nmx = const.tile([8, 1], F32)
            nc.vector.tensor_reduce(nmx[:], lgs[:], mybir.AxisListType.X, Alu.max,
                                    negate=True)
            ex = const.tile([8, 3], F32)
            smv = const.tile([8, 1], F32)
            nc.scalar.activation(ex[:], lgs[:], Act.Exp, bias=nmx[:], scale=1.0,
                                 accum_out=smv[:])
            ri = const.tile([8, 1], F32)
            nc.vector.reciprocal(ri[:], smv[:])
            pr = const.tile([8, 3], F32)
            nc.vector.tensor_scalar(pr[:], ex[:], ri[:], None, Alu.mult)
            nc.sync.dma_start(out[:], pr[:])

    nc.compile()
    return nc


def _extract_weights(inputs):
    f = lambda a: [float(v) for v in np.asarray(a).reshape(-1)]
    return dict(
        w00=f(inputs["c0_w"])[0], w01=f(inputs["c0_w"])[1], b0=f(inputs["c0_b"])[0],
        w2=f(inputs["c2_w"]), b2=f(inputs["c2_b"])[0],
        w4=f(inputs["c4_w"]), b4=f(inputs["c4_b"])[0],
        w6=f(inputs["c6_w"]), b6=f(inputs["c6_b"])[0],
        w8=f(inputs["c8_w"]), b8=f(inputs["c8_b"])[0],
        cls_b=f(inputs["cls_b"]),
    )


def _run(inputs, trace=False, trace_kwargs=None):
    w = _extract_weights(inputs)
    key = tuple(np.asarray(
        [w["w00"], w["w01"], w["b0"]] + w["w2"] + [w["b2"]] + w["w4"] + [w["b4"]]
        + w["w6"] + [w["b6"]] + w["w8"] + [w["b8"]] + w["cls_b"], np.float64
    ).tobytes())
    if key not in _CACHE:
        _CACHE[key] = _build(w)
    nc = _CACHE[key]

    x = np.ascontiguousarray(np.asarray(inputs["x"], dtype=np.float32))
    xf = x.reshape(BS * NN, T)
    cls_w = np.asarray(inputs["cls_w"], dtype=np.float32)       # (3, 1024)
    clsT = np.empty((128, 24), np.float32)
    for c in range(C):
        clsT[:, c * 3:(c + 1) * 3] = cls_w[:, c * 128:(c + 1) * 128].T

    rows_per_core = BS * NN // N_CORES
    in_maps = [
        {"x": np.ascontiguousarray(xf[i * rows_per_core:(i + 1) * rows_per_core]),
         "clswt": clsT}
        for i in range(N_CORES)
    ]
    res = run_bass_kernel_spmd(nc, in_maps, list(range(N_CORES)), trace=trace,
                               **(trace_kwargs or {}))
    out = np.concatenate([np.asarray(res.results[i]["out"]) for i in range(N_CORES)],
                         axis=0).astype(np.float32)
    return out, res


def kernel(**inputs):
    out, _ = _run(inputs, trace=False)
    return out
